# revision 4
# baseline (speedup 1.0000x reference)
"""Trainium2 Bass kernel for HandDecoder-style GNN message passing, v2.

Math (per batch element b, N=128 nodes):
  f = relu(MLP3([feature, coords]))                       # [N, C1=32]
  a = coords @ kw1 (+ kb1)                                # [N, 8]
  t1[i,j,h] = relu(a[j,h] + kb1[h] - a[i,h])              # [N,N,8]
  t2[i,j,k] = relu(sum_h t1[i,j,h] kw2[h,k] + kb2[k])     # [N,N,16]
  g[j,k,d]  = sum_c f[j,c] kw3p[c, k*16+d]                # [N,16,16]
  out[i,d]  = relu(sum_{j,k} t2[i,j,k] g[j,k,d] + bias2[d])
  bias2[d]  = sum_c (sum_j f[j,c]) kb3r[c,d]

v2 performance structure (vs v1's 212 fp32 matmuls at ~430ns each):
  - bf16 matmuls (1 cyc/row, single weight load) with wide moving
    operands: t1 = 8 matmuls of 512 free, t2 = 16 of 512, decode in
    fp32r (1 cyc/row at free>=256).
  - final contraction via "junk block" trick: J[(b,d),(b',i)] =
    sum_s g_blk_s^T @ t2_s accumulated in one PSUM tile; only the
    b==b' diagonal blocks are kept. 16 matmuls of 512 free instead of
    68 tiny ones (weight-load bound).
  - g permuted to [(jlh,k), (b,s,d)] via a DRAM bounce whose gather
    descriptors (32B) spread over all 16 DMA engines, overlapped with
    t1/t2 compute.
Data-parallel over batch: 4 batch elements per core, 8 cores.
"""

import sys
import numpy as np
import ml_dtypes

for _p in ("/opt/trn_rl_repo",):
    if _p not in sys.path:
        sys.path.insert(0, _p)

import concourse.bass as bass
import concourse.bacc as bacc
import concourse.mybir as mybir
import concourse.tile as tile
from concourse.bass_utils import run_bass_kernel_spmd

B, N = 32, 128
C0, C1, C2 = 64, 32, 16
NCORES = 8
BPC = B // NCORES          # 4 batch elements per core
F32 = mybir.dt.float32
F32R = mybir.dt.float32r
BF16 = mybir.dt.bfloat16
RELU = mybir.ActivationFunctionType.Relu
COPY = mybir.ActivationFunctionType.Copy
ADD = mybir.AluOpType.add
MAX = mybir.AluOpType.max
NPBF = ml_dtypes.bfloat16

_CACHED_NC = None


def build_nc():
    nc = bacc.Bacc("TRN2", target_bir_lowering=False, debug=False,
                   num_devices=NCORES)

    # ---- DRAM tensors ----
    # cst12: rhs12 [12,512] (rows 0-7 device aneg, 8-11 host b-ind) at
    #   cols 0:512; c4T [4,512] at cols 512:1024; kw1b4 [4,8] at
    #   1024:1032; kw1n4 [4,8] at 1032:1040; ones [1,512] at 1040:1552.
    cst12_d = nc.dram_tensor("cst12", [12, 1552], BF16, kind="ExternalInput").ap()
    # cst32: lhsT_t1 [12,1024] (rows 0-7 host selectors, rows 8-11
    #   device a2b) at cols 0:1024; kw3p [32,256] at cols 1536:1792.
    cst32_d = nc.dram_tensor("cst32", [32, 1792], BF16, kind="ExternalInput").ap()
    # kwbd: block-diag kw2, half0 cols 0:128, half1 cols 128:256.
    kwbd_d = nc.dram_tensor("kwbd", [128, 256], BF16, kind="ExternalInput").ap()
    # wb (fp32r): dw1 [67,0:32], dw2 [32,32:48], dw3 [16,48:80]
    wba_d = nc.dram_tensor("wba", [34, 80], F32R, kind="ExternalInput").ap()
    wbb_d = nc.dram_tensor("wbb", [33, 80], F32R, kind="ExternalInput").ap()
    # wc (fp32): db1 col 0, db2 col 1, db3 col 2, kb2t col 3, kb3r 4:20
    wc_d = nc.dram_tensor("wc", [128, 20], F32, kind="ExternalInput").ap()
    xTa_d = nc.dram_tensor("xTa", [34, BPC * N], F32R, kind="ExternalInput").ap()
    xTb_d = nc.dram_tensor("xTb", [33, BPC * N], F32R, kind="ExternalInput").ap()
    out_d = nc.dram_tensor("out", [C2, BPC * N], F32, kind="ExternalOutput").ap()
    a_dram = nc.dram_tensor("ascr", [4096], BF16).ap()
    g_dram = nc.dram_tensor("gscr", [131072], BF16).ap()

    with tile.TileContext(nc) as tc:
        with (
            tc.tile_pool(name="work", bufs=1) as wpool,
            tc.tile_pool(name="psW", bufs=1, space=bass.MemorySpace.PSUM) as psW,
            tc.tile_pool(name="psA", bufs=2, space=bass.MemorySpace.PSUM) as psA,
            tc.tile_pool(name="psB", bufs=2, space=bass.MemorySpace.PSUM) as psB,
            tc.tile_pool(name="psJ", bufs=1, space=bass.MemorySpace.PSUM) as psJ,
        ):
            cpool = wpool
            # ---- input DMAs (sync queue) ----
            cst12 = cpool.tile([12, 1552], BF16, tag="cst12")
            cst32 = cpool.tile([32, 1792], BF16, tag="cst32")
            kwbd = cpool.tile([128, 256], BF16, tag="kwbd")
            wb = cpool.tile([67, 80], F32R, tag="wb")
            wc = cpool.tile([128, 20], F32, tag="wc")
            xT_s = cpool.tile([67, BPC * N], F32R, tag="xT")
            # PE clock warm-up: ~4us of dependency-free matmuls during the
            # input-DMA head flips the HAM clock gate to 2.4 GHz before the
            # real matmul stream begins.
            wtile = wpool.tile([2, 640], BF16, tag="warm")
            nc.vector.memset(wtile[:], 0.0)
            w_ps = psW.tile([8, 512], F32, tag="w")
            for _ in range(8):
                nc.tensor.matmul(w_ps[:], wtile[0:2, 0:8], wtile[0:2, 128:640])

            # xT/wb split into two separately-allocated DRAM tensors:
            # descriptor->DMA-engine assignment follows the tensor's DRAM
            # address, so two allocations can land on different channels
            # instead of serializing one engine.
            nc.sync.dma_start(cst12[:], cst12_d)
            nc.sync.dma_start(xT_s[0:34, :], xTa_d)
            nc.sync.dma_start(xT_s[34:67, :], xTb_d)
            nc.sync.dma_start(cst32[:], cst32_d)
            nc.scalar.dma_start(wb[0:34, :], wba_d)
            nc.scalar.dma_start(wb[34:67, :], wbb_d)
            nc.scalar.dma_start(wc[:], wc_d)
            nc.gpsimd.dma_start(kwbd[:], kwbd_d)

            rhs12 = cst12[0:12, 0:512]
            c4T = cst12[0:4, 512:1024]
            kw1b4 = cst12[0:4, 1024:1032]
            kw1n4 = cst12[0:4, 1032:1040]
            ones512 = cst12[0:1, 1040:1552]
            lhsT_t1 = cst32[0:12, 0:1024]
            kw3p = cst32[0:32, 1536:1792]
            xT = xT_s[0:67, 0:512]
            dw1 = wb[0:67, 0:32]
            dw2 = wb[0:32, 32:48]
            dw3 = wb[0:16, 48:80]
            db1 = wc[0:32, 0:1]
            db2 = wc[0:16, 1:2]
            db3 = wc[0:32, 2:3]
            kb2t = wc[0:128, 3:4]
            kb3r = wc[0:32, 4:20]

            # ---- a-stage (bf16): aneg = -(coords@kw1) -> rhs12 rows 0-7
            aneg_ps = psA.tile([8, 512], F32, tag="pa")
            nc.tensor.matmul(aneg_ps[:], kw1n4, c4T)
            nc.scalar.activation(cst12[0:8, 0:512], aneg_ps[:], COPY)

            # a2b[j,(b,h)] = coords@kw1 + kb1, row-major for the bounce
            a2b_ps = psA.tile([128, 32], F32, tag="pa")
            for b in range(BPC):
                nc.tensor.matmul(a2b_ps[:, b * 8:(b + 1) * 8],
                                 c4T[0:4, b * N:(b + 1) * N], kw1b4)
            a2b_sb = wpool.tile([128, 32], BF16, tag="a2b")
            nc.vector.tensor_copy(a2b_sb[:], a2b_ps[:])
            # bounce: a_dram[j*32+b*8+h]; read back into lhsT_t1 rows 8-11
            nc.gpsimd.dma_start(a_dram, a2b_sb[:])
            src_a = a_dram.rearrange("(cj b h) -> b cj h", b=4, h=8)
            dst_a = cst32[8:12, 0:1024].rearrange("p (cj h) -> p cj h", h=8)
            nc.gpsimd.dma_start(dst_a, src_a)

            # ---- decode MLP (fp32r wide matmuls) ----
            h1 = wpool.tile([32, 512], F32R, tag="h1")
            h2 = wpool.tile([16, 512], F32R, tag="h2")
            fT = wpool.tile([32, 512], BF16, tag="fT")
            d1_ps = psB.tile([32, 512], F32, tag="pb")
            nc.tensor.matmul(d1_ps[:], dw1, xT)
            nc.scalar.activation(h1[:], d1_ps[:], RELU, bias=db1)
            d2_ps = psA.tile([16, 512], F32, tag="pa")
            nc.tensor.matmul(d2_ps[:], dw2, h1[:])
            nc.vector.tensor_scalar(h2[:], d2_ps[:], db2, 0.0, ADD, MAX)
            d3_ps = psB.tile([32, 512], F32, tag="pb")
            nc.tensor.matmul(d3_ps[:], dw3, h2[:])
            nc.scalar.activation(fT[:], d3_ps[:], RELU, bias=db3)

            # ---- g stage: g_rm[j,(k,d)] per b, bf16, bounce to
            #      g_all[(jlh,k), (s,b,d)] ----
            g_sb = []
            for p in range(2):          # b-pairs
                g_ps = psB.tile([128, 512], F32, tag="pb")
                for q in range(2):
                    bq = p * 2 + q
                    nc.tensor.matmul(g_ps[:, q * 256:(q + 1) * 256],
                                     fT[0:32, bq * N:(bq + 1) * N], kw3p)
                gs = wpool.tile([128, 512], BF16, tag=f"gsb{p}")
                if p == 0:
                    nc.scalar.activation(gs[:], g_ps[:], COPY)
                else:
                    nc.vector.tensor_copy(gs[:], g_ps[:])
                g_sb.append(gs)
            # bounce: write g_dram[b*32768 + j*256 + (k*16+d)] per b-pair,
            # then immediately read that pair's columns back into the padded
            # g_all layout (col = s*128 + b*32 + d; d=16..31 junk cols feed
            # never-read J partitions).  Pipelining pairs overlaps the
            # write->read round trip.
            g_all = wpool.tile([128, 2048], BF16, tag="gall")
            # zero only the pad columns (d=16..31 of each (s,b) block) on
            # DVE: keeps the gpsimd DMA queue free for the g bounce.
            g_all4 = g_all[:].rearrange("p (s b dp) -> p s b dp",
                                        s=16, b=4, dp=32)
            nc.vector.memset(g_all4[:, :, :, 16:32], 0.0)
            src_b = g_dram.rearrange("(b s p d) -> b p s d",
                                     b=4, s=16, p=128, d=16)
            g_all3 = g_all[:].rearrange("p (s c) -> p s c", s=16, c=128)
            for p in range(2):
                dst = g_dram[p * 65536:(p + 1) * 65536].rearrange(
                    "(b j kd) -> j b kd", b=2, kd=256)
                nc.gpsimd.dma_start(dst, g_sb[p][:].rearrange(
                    "p (b kd) -> p b kd", b=2))
                for q in range(2):
                    b = p * 2 + q
                    nc.sync.dma_start(g_all3[:, :, b * 32:b * 32 + 16],
                                      src_b[b])

            # ---- bias2[b,d] = sum_c F[b,c] kb3r[c,d], F = rowsum f ----
            F_sb = wpool.tile([32, 4], F32, tag="F")
            nc.vector.tensor_reduce(
                F_sb[:].rearrange("p (b o) -> p b o", o=1),
                fT[:].rearrange("p (b j) -> p b j", b=4),
                mybir.AxisListType.X, ADD)
            b2_ps = psA.tile([1, 64], F32, tag="pa")
            for b in range(BPC):
                nc.tensor.matmul(b2_ps[0:1, b * 16:(b + 1) * 16],
                                 F_sb[0:32, b:b + 1], kb3r)
            b2_sb = wpool.tile([1, 128], BF16, tag="b2")
            nc.gpsimd.memset(b2_sb[:], 0.0)
            nc.vector.tensor_copy(
                b2_sb[:].rearrange("p (b dp) -> p b dp", b=4, dp=32)[:, :, 0:16],
                b2_ps[:].rearrange("p (b d) -> p b d", b=4, d=16))

            # ---- t1: 8 matmuls [12,128]x[12,512] -> relu -> bf16 ----
            t1_sb = []
            for c in range(8):
                t1_ps = psA.tile([128, 512], F32, tag="pa")
                nc.tensor.matmul(t1_ps[:],
                                 lhsT_t1[0:12, c * 128:(c + 1) * 128], rhs12)
                ts = wpool.tile([128, 512], BF16, tag=f"t1sb{c}")
                if c % 2 == 0:
                    nc.scalar.activation(ts[:], t1_ps[:], RELU)
                else:
                    nc.vector.tensor_scalar(ts[:], t1_ps[:], 0.0, None, MAX)
                t1_sb.append(ts)

            # ---- t2: 16 matmuls kw2bd-half x t1 tile -> relu+kb2 ----
            # tile (c, hf): rows (jlh,k), cols (b,i); j = c*16 + hf*8 + jlh
            t2_sb = []
            for c in range(8):
                t2_ps = psB.tile([128, 1024], F32, tag="pb")
                for hf in range(2):
                    nc.tensor.matmul(t2_ps[:, hf * 512:(hf + 1) * 512],
                                     kwbd[:, hf * 128:(hf + 1) * 128],
                                     t1_sb[c][:])
                ts = wpool.tile([128, 1024], BF16, tag=f"t2sb{c}")
                if c % 2 == 0:
                    nc.scalar.activation(ts[:], t2_ps[:], RELU, bias=kb2t)
                else:
                    nc.vector.tensor_scalar(ts[:], t2_ps[:], kb2t, 0.0,
                                            ADD, MAX)
                t2_sb.append(ts)

            # ---- final: J[(b,d),(b',i)] = sum_s g_blk_s^T @ t2_s + bias
            J_ps = psJ.tile([128, 512], F32, tag="J")
            for c in range(8):
                for hf in range(2):
                    s = c * 2 + hf
                    nc.tensor.matmul(J_ps[:],
                                     g_all[:, s * 128:(s + 1) * 128],
                                     t2_sb[c][:, hf * 512:(hf + 1) * 512],
                                     start=(s == 0), stop=False)
            nc.tensor.matmul(J_ps[:], b2_sb[0:1, 0:128], ones512,
                             start=False, stop=True)

            # diagonal blocks -> relu -> O_sb[d, (b,i)]
            O_sb = wpool.tile([16, 512], F32, tag="O")
            for b in range(BPC):
                blk_in = J_ps[b * 32:b * 32 + 16, b * 128:(b + 1) * 128]
                blk_out = O_sb[0:16, b * 128:(b + 1) * 128]
                if b % 2 == 0:
                    nc.scalar.activation(blk_out, blk_in, RELU)
                else:
                    nc.vector.tensor_scalar(blk_out, blk_in, 0.0, None, MAX)
            nc.sync.dma_start(out_d, O_sb[:])

    nc.compile()
    return nc


def _host_inputs(feature, coordinates_v, dw1, db1, dw2, db2, dw3, db3,
                 kw1, kb1, kw2, kb2, kw3, kb3):
    """Per-core input maps. Pure layout transforms, no FLOPs."""
    f32 = np.float32

    # cst32 (core-independent): selector rows + kw3p
    cst32 = np.zeros((32, 1792), f32)
    cols = np.arange(1024)
    for r in range(8):
        cst32[r, 0:1024] = (cols % 8 == r)
    cst32[0:32, 1536:1792] = (
        kw3.reshape(16, 32, 16).transpose(1, 0, 2).reshape(32, 256))
    cst32 = cst32.astype(NPBF)

    # kwbd (core-independent)
    bd = np.zeros((128, 256), f32)
    for hf in range(2):
        for jlh in range(8):
            jl = hf * 8 + jlh
            bd[jl * 8:(jl + 1) * 8,
               hf * 128 + jlh * 16: hf * 128 + (jlh + 1) * 16] = kw2
    kwbd = bd.astype(NPBF)

    # weights (core-independent)
    wbh = np.zeros((67, 80), f32)
    wbh[0:67, 0:32] = dw1
    wbh[0:32, 32:48] = dw2
    wbh[0:16, 48:80] = dw3
    wch = np.zeros((128, 20), f32)
    wch[0:32, 0] = db1
    wch[0:16, 1] = db2
    wch[0:32, 2] = db3
    wch[:, 3] = np.tile(kb2, 8)
    wch[0:32, 4:20] = kb3.reshape(32, 16)

    in_maps = []
    for core in range(NCORES):
        fe = feature[core * BPC:(core + 1) * BPC]          # [4, 64]
        co = coordinates_v[core * BPC:(core + 1) * BPC]    # [4, 128, 3]

        cst12 = np.zeros((12, 1552), f32)
        ci = np.arange(512)
        for r in range(4):
            cst12[8 + r, 0:512] = (ci // 128 == r)         # b-indicators
        for b in range(BPC):
            cst12[0:3, 512 + b * N: 512 + (b + 1) * N] = co[b].T
        cst12[3, 512:1024] = 1.0
        cst12[0:3, 1024:1032] = kw1
        cst12[3, 1024:1032] = kb1
        cst12[0:3, 1032:1040] = -kw1
        cst12[0, 1040:1552] = 1.0                          # row 0: ones
        cst12 = cst12.astype(NPBF)

        xTh = np.empty((67, BPC * N), f32)
        for b in range(BPC):
            xTh[0:64, b * N:(b + 1) * N] = fe[b][:, None]
            xTh[64:67, b * N:(b + 1) * N] = co[b].T
        in_maps.append({"cst12": cst12, "cst32": cst32, "kwbd": kwbd,
                        "wba": np.ascontiguousarray(wbh[0:34]),
                        "wbb": np.ascontiguousarray(wbh[34:67]),
                        "wc": wch,
                        "xTa": np.ascontiguousarray(xTh[0:34]),
                        "xTb": np.ascontiguousarray(xTh[34:67])})
    return in_maps


def kernel(**inputs):
    global _CACHED_NC
    if _CACHED_NC is None:
        _CACHED_NC = build_nc()
    nc = _CACHED_NC
    in_maps = _host_inputs(
        np.asarray(inputs["feature"]), np.asarray(inputs["coordinates_v"]),
        np.asarray(inputs["dw1"]), np.asarray(inputs["db1"]),
        np.asarray(inputs["dw2"]), np.asarray(inputs["db2"]),
        np.asarray(inputs["dw3"]), np.asarray(inputs["db3"]),
        np.asarray(inputs["kw1"]), np.asarray(inputs["kb1"]),
        np.asarray(inputs["kw2"]), np.asarray(inputs["kb2"]),
        np.asarray(inputs["kw3"]), np.asarray(inputs["kb3"]))
    res = run_bass_kernel_spmd(nc, in_maps, list(range(NCORES)))
    out = np.empty((B, N, C2), np.float32)
    for c in range(NCORES):
        # per-core out is [C2(d), BPC(b)*N(i)]
        o = res.results[c]["out"].reshape(C2, BPC, N)
        out[c * BPC:(c + 1) * BPC] = o.transpose(1, 2, 0)
    return out
